# revision 1
# baseline (speedup 1.0000x reference)
"""Trainium2 Bass kernel: single-channel 15x15 cross-correlation (pad=1,
stride=1) of a 4096x4096 fp32 image, + scalar bias.

Strategy
--------
Output is 4084x4084 (padded here to a 4104x4096 grid).  The conv is computed
as banded ("Toeplitz") matmuls on the tensor engine: for each of the 15
kernel columns dj, a stationary band matrix A_dj[k, m] = W[k-m, dj]
(k in [0,128) input rows, m in [0,114) output rows) multiplies a
column-shifted slice of the input tile, accumulating all 15 dj into one
PSUM tile [114, 512].  All operands are float32r (fp32 bits, fp22 multiply
inside the PE) which runs at full PE rate for free dim >= 256.

Work is tiled as 36 row-blocks (114 output rows each) x 8 column chunks
(512 cols) = 288 tiles, split evenly across 8 NeuronCores: each core owns
4 consecutive row-blocks (32 tiles) plus half of one shared row-block
(4 tiles) = 36 tiles = 540 matmuls.  Halos are included in each core's
host-side input slices, so no collectives are needed.

Input DMA is chunk-granular ([128, 526] per tile) so the tensor engine
starts ~3us into the kernel and never stalls on loads.
"""

import os

import numpy as np

KH = KW = 15
PAD = 1
H = W = 4096
OUT = H + 2 * PAD - KH + 1  # 4084
NCORES = 8
BLK_M = 114  # output rows per row-block (128 - (KH - 1))
NBLK = 36  # global row-blocks: 36*114 = 4104 >= 4084
OWN_BLKS = 4  # row-blocks fully owned by each core (32 total)
SH_CHUNKS = 4  # chunks of the shared row-block each core handles
CHUNK = 512  # output cols per PSUM tile
NCHUNK = 8  # 8*512 = 4096 >= 4084
XW = CHUNK + KW - 1  # 526: input cols per chunk tile
OWN_ROWS = OWN_BLKS * BLK_M  # 456 output rows owned per core
XROWS_OWN = OWN_ROWS + KH - 1  # 470 input rows per core (own part)
XCOLS = NCHUNK * CHUNK + KW - 1  # 4110 input cols
SH_XCOLS = SH_CHUNKS * CHUNK + KW - 1  # 2062 input cols (shared part)
XBIG_ROWS = NBLK * BLK_M + KH - 1  # 4118 padded input rows
SH_BLK0 = 32  # first shared row-block index

LAST_RESULT = None  # BassKernelResults of the most recent run (for test.py)


def _patch_drain():
    """walrus's CTRL_NO instruction struct holds very few semaphore waits;
    Tile's kernel-tail drain aggregates one wait per logical processor and
    overflows it.  Spread the waits across 1-wait-per-nop SP instructions."""
    import concourse.mybir as mybir
    import concourse.tile as tile
    from concourse.vector_clock import ScopedClock

    def _split_drain_and_barrier(self, tick_clock, wait_clock):
        nc = self.nc
        probe = nc.sync.nop(nofuse=True)
        wait_clock.add_sem_waits(
            probe.ins, ScopedClock({None: tick_clock.global_clock})
        )
        si = probe.ins.sync_info
        if si is not None and len(si.on_wait) > 1:
            waits = list(si.on_wait)
            probe.ins.sync_info = mybir.SyncInfo(
                on_wait=waits[:1], on_update=list(si.on_update)
            )
            for w in waits[1:]:
                extra = nc.sync.nop(nofuse=True)
                extra.ins.sync_info = mybir.SyncInfo(on_wait=[w], on_update=[])
        nc.sync.drain()
        # The stock exit path does barrier -> semaphore cleanup -> barrier
        # (~8us).  This NEFF executes once per load, so leftover semaphore
        # values don't matter: skip the cleanup, keep only the drain (which
        # carries the waits that guarantee all DMAs have landed).
        assert self.sems is not None
        popped = nc._tile_sem_poison_stack.pop()
        assert popped is self._sem_poison

    tile.TileContext._drain_and_barrier = _split_drain_and_barrier


def _split_multi_waits(nc):
    """This compiler's TPB instruction structs hold only one sync-wait slot
    (walrus setupSyncWait rejects more).  Tile sometimes assigns 2+ waits
    (DMA completion + slot release) to one instruction; split the excess onto
    same-engine nops inserted immediately before it."""
    import concourse.mybir as mybir

    for fn in nc.m.functions:
        for bb in fn.blocks:
            insts = list(bb.instructions)
            out = []
            changed = False
            for inst in insts:
                si = inst.sync_info
                if (
                    not isinstance(inst, mybir.InstNoOp)
                    and si is not None
                    and len(si.on_wait) > 1
                ):
                    waits = list(si.on_wait)
                    for w in waits[:-1]:
                        nop = mybir.InstNoOp(
                            name=nc.get_next_instruction_name(),
                            engine=inst.engine,
                            bass_nofuse=True,
                            sync_info=mybir.SyncInfo(on_wait=[w], on_update=[]),
                        )
                        nc.register_instruction(nop)
                        out.append(nop)
                    inst.sync_info = mybir.SyncInfo(
                        on_wait=[waits[-1]], on_update=list(si.on_update)
                    )
                    changed = True
                out.append(inst)
            if changed:
                bb.instructions = out


def _make_bands(weight):
    """bands[k, dj*BLK_M + m] = W[k-m, dj] for k-m in [0, KH)."""
    A = np.zeros((128, KW, BLK_M), np.float32)
    idx = np.arange(BLK_M)
    for dj in range(KW):
        for di in range(KH):
            A[idx + di, dj, idx] = weight[di, dj]
    return np.ascontiguousarray(A.reshape(128, KW * BLK_M))


def _build_program(bias_val):
    import concourse.bass as bass
    import concourse.mybir as mybir
    import concourse.tile as tile

    _patch_drain()
    f32r = mybir.dt.float32r
    f32 = mybir.dt.float32

    nc = bass.Bass()
    x_own = nc.declare_dram_parameter("x_own", [XROWS_OWN, XCOLS], f32r, isOutput=False)
    x_sh = nc.declare_dram_parameter("x_sh", [128, SH_XCOLS], f32r, isOutput=False)
    bands = nc.declare_dram_parameter("bands", [128, KW * BLK_M], f32r, isOutput=False)
    out_own = nc.declare_dram_parameter(
        "out_own", [OWN_ROWS, NCHUNK * CHUNK], f32, isOutput=True
    )
    out_sh = nc.declare_dram_parameter(
        "out_sh", [BLK_M, SH_CHUNKS * CHUNK], f32, isOutput=True
    )

    with tile.TileContext(nc) as tc:
        with (
            tc.tile_pool(name="const", bufs=1) as constp,
            tc.tile_pool(name="xp", bufs=8) as xp,
            tc.tile_pool(name="psum", bufs=4, space="PSUM") as psp,
            tc.tile_pool(name="op", bufs=4) as outp,
        ):
            # 15 separate band tiles so the first matmul only waits for the
            # first 58 KB load, not the whole 875 KB bands tensor.  Bands and
            # output stores ride the Activation engine's HWDGE ring; the SP
            # ring is reserved for input tiles so the first x chunk is the
            # first transfer in its queue.
            bts = []
            for dj in range(KW):
                bt = constp.tile([128, BLK_M], f32r, tag=f"band{dj}")
                nc.scalar.dma_start(out=bt[:, :], in_=bands[:, BLK_M * dj : BLK_M * (dj + 1)])
                bts.append(bt)

            def do_tile(src, r0, c0, dst, dr0, dc0):
                """One [114, 512] output tile: 15 banded matmuls + evac."""
                xt = xp.tile([128, XW], f32r, tag="xt")
                nc.sync.dma_start(out=xt[:, :], in_=src[r0 : r0 + 128, c0 : c0 + XW])
                ps = psp.tile([BLK_M, CHUNK], f32, tag="ps")
                for dj in range(KW):
                    nc.tensor.matmul(
                        ps[:, :],
                        bts[dj][:, :],
                        xt[:, dj : dj + CHUNK],
                        start=(dj == 0),
                        stop=(dj == KW - 1),
                    )
                ot = outp.tile([BLK_M, CHUNK], f32, tag="ot")
                nc.vector.tensor_scalar_add(ot[:, :], ps[:, :], bias_val)
                nc.scalar.dma_start(
                    out=dst[dr0 : dr0 + BLK_M, dc0 : dc0 + CHUNK], in_=ot[:, :]
                )

            for b in range(OWN_BLKS):
                for q in range(NCHUNK):
                    do_tile(x_own, BLK_M * b, CHUNK * q, out_own, BLK_M * b, CHUNK * q)
            for q in range(SH_CHUNKS):
                do_tile(x_sh, 0, CHUNK * q, out_sh, 0, CHUNK * q)

    _split_multi_waits(nc)
    return nc


def kernel(x, weight, bias):
    global LAST_RESULT
    from concourse.bass_utils import run_bass_kernel_spmd

    x = np.ascontiguousarray(np.asarray(x, dtype=np.float32))
    weight = np.asarray(weight, dtype=np.float32)
    bias = np.asarray(bias, dtype=np.float32)

    # Host-side zero padding: PAD on top/left, plus enough extra rows/cols
    # that every core's fixed-size slice stays in bounds.
    xbig = np.zeros((XBIG_ROWS, XCOLS), np.float32)
    xbig[PAD : PAD + H, PAD : PAD + W] = x
    bands = _make_bands(weight)

    nc = _build_program(float(bias[0]))
    in_maps = []
    for c in range(NCORES):
        sh_blk = SH_BLK0 + c // 2
        sh_col = (SH_CHUNKS * CHUNK) * (c % 2)
        in_maps.append(
            {
                "x_own": np.ascontiguousarray(
                    xbig[OWN_ROWS * c : OWN_ROWS * c + XROWS_OWN]
                ),
                "x_sh": np.ascontiguousarray(
                    xbig[BLK_M * sh_blk : BLK_M * sh_blk + 128, sh_col : sh_col + SH_XCOLS]
                ),
                "bands": bands,
            }
        )
    res = run_bass_kernel_spmd(
        nc,
        in_maps,
        list(range(NCORES)),
        trace=bool(os.environ.get("CONV_TRACE")),
    )
    LAST_RESULT = res

    full = np.empty((NBLK * BLK_M, NCHUNK * CHUNK), np.float32)
    for c in range(NCORES):
        r = res.results[c]
        full[OWN_ROWS * c : OWN_ROWS * (c + 1)] = r["out_own"]
        sh_blk = SH_BLK0 + c // 2
        sh_col = (SH_CHUNKS * CHUNK) * (c % 2)
        full[
            BLK_M * sh_blk : BLK_M * (sh_blk + 1), sh_col : sh_col + SH_CHUNKS * CHUNK
        ] = r["out_sh"]
    return np.ascontiguousarray(full[:OUT, :OUT]).astype(np.float32)



# revision 2
# speedup vs baseline: 2.1016x; 2.1016x over previous
"""Trainium2 Bass kernel: single-channel 15x15 cross-correlation (pad=1,
stride=1) of a 4096x4096 fp32 image, + scalar bias.

Strategy (v2: 2D-patch packing, 6 matmul passes per 128-pixel block)
--------------------------------------------------------------------
The image is space-to-depth'd on the host: partition dim packs a 16x8
(row x col) patch, so SBUF tile XR[band][(r, co), g] = xpad[16*band + r,
8*g + co].  An output block of 16x8 = 128 pixels out[(io, jo)] at column
group n accumulates 6 matmuls (a in {0,1} row-patches x b in {0,1,2}
col-shifts): stationary A_ab[(r, co), (io, jo)] = W[16a + r - io,
8b + co - jo], moving operand XR[t+a][:, n+b].  Each (di, dj) weight tap
appears in exactly one (a, b, r, co), so 6 passes replace the 15 banded
passes of the naive Toeplitz scheme: 2.5x fewer tensor-engine cycles
(the 6-pass count is optimal: a block needs 660 distinct inputs and a
stream column carries at most 128).

Work per core: 32 bands (512 output rows) x 512 column groups -> 192
matmuls of [K=128, M=128] x [128, 512] in fp32r (full PE rate for free
dim >= 256).  Host pre-pads/reshapes inputs and un-shuffles outputs, so
all DMA is contiguous [128, 2KB+] tiles; halos ride in each core's input
slice, no collectives.
"""

import os

import numpy as np

KH = KW = 15
PAD = 1
H = W = 4096
OUT = H + 2 * PAD - KH + 1  # 4084
NCORES = 8
BR = 16  # output rows per band (and patch rows)
BC = 8  # col-group width (and patch cols)
NBAND = 32  # bands per core -> 512 output rows per core
NG = 512  # output col groups per stream (512*8 = 4096 >= 4084 cols)
GROUPS = NG + 4  # col groups per XR tile (2 extra for b-shift, pad to 516)
NPASS = 6  # 2 row-patches x 3 col-shifts
ROWS_PC = NBAND * BR  # 512 output rows per core
XR_BANDS = NBAND + 1  # input bands per core (one extra for the halo)
XPAD_R = NCORES * ROWS_PC + BR  # 4112 padded input rows
XPAD_C = GROUPS * BC  # 4128 padded input cols

LAST_RESULT = None  # BassKernelResults of the most recent run (for test.py)


def _patch_drain():
    """walrus's CTRL_NO instruction struct holds very few semaphore waits;
    Tile's kernel-tail drain aggregates one wait per logical processor and
    overflows it.  Spread the waits across 1-wait-per-nop SP instructions."""
    import concourse.mybir as mybir
    import concourse.tile as tile
    from concourse.vector_clock import ScopedClock

    def _split_drain_and_barrier(self, tick_clock, wait_clock):
        nc = self.nc
        probe = nc.sync.nop(nofuse=True)
        wait_clock.add_sem_waits(
            probe.ins, ScopedClock({None: tick_clock.global_clock})
        )
        si = probe.ins.sync_info
        if si is not None and len(si.on_wait) > 1:
            waits = list(si.on_wait)
            probe.ins.sync_info = mybir.SyncInfo(
                on_wait=waits[:1], on_update=list(si.on_update)
            )
            for w in waits[1:]:
                extra = nc.sync.nop(nofuse=True)
                extra.ins.sync_info = mybir.SyncInfo(on_wait=[w], on_update=[])
        nc.sync.drain()
        # The stock exit path does barrier -> semaphore cleanup -> barrier
        # (~8us).  This NEFF executes once per load, so leftover semaphore
        # values don't matter: skip the cleanup, keep only the drain (which
        # carries the waits that guarantee all DMAs have landed).
        assert self.sems is not None
        popped = nc._tile_sem_poison_stack.pop()
        assert popped is self._sem_poison

    tile.TileContext._drain_and_barrier = _split_drain_and_barrier


def _split_multi_waits(nc):
    """This compiler's TPB instruction structs hold only one sync-wait slot
    (walrus setupSyncWait rejects more).  Tile sometimes assigns 2+ waits
    (DMA completion + slot release) to one instruction; split the excess onto
    same-engine nops inserted immediately before it."""
    import concourse.mybir as mybir

    for fn in nc.m.functions:
        for bb in fn.blocks:
            insts = list(bb.instructions)
            out = []
            changed = False
            for inst in insts:
                si = inst.sync_info
                if (
                    not isinstance(inst, mybir.InstNoOp)
                    and si is not None
                    and len(si.on_wait) > 1
                ):
                    waits = list(si.on_wait)
                    for w in waits[:-1]:
                        nop = mybir.InstNoOp(
                            name=nc.get_next_instruction_name(),
                            engine=inst.engine,
                            bass_nofuse=True,
                            sync_info=mybir.SyncInfo(on_wait=[w], on_update=[]),
                        )
                        nc.register_instruction(nop)
                        out.append(nop)
                    inst.sync_info = mybir.SyncInfo(
                        on_wait=[waits[-1]], on_update=list(si.on_update)
                    )
                    changed = True
                out.append(inst)
            if changed:
                bb.instructions = out


def _make_stationaries(weight):
    """A[(r, co), idx*128 + (io*8 + jo)] = W[16a + r - io, 8b + co - jo]
    for idx = 3a + b, wherever the taps are in [0, 15)."""
    A = np.zeros((2, 3, BR, BC, BR, BC), np.float32)  # [a, b, r, co, io, jo]
    for a in range(2):
        for b in range(3):
            for io in range(BR):
                for jo in range(BC):
                    for di in range(KH):
                        r = io + di - 16 * a
                        if not 0 <= r < BR:
                            continue
                        for dj in range(KW):
                            co = jo + dj - 8 * b
                            if 0 <= co < BC:
                                A[a, b, r, co, io, jo] = weight[di, dj]
    A = A.reshape(NPASS, BR * BC, BR * BC).transpose(1, 0, 2)  # [k, idx, m]
    return np.ascontiguousarray(A.reshape(BR * BC, NPASS * BR * BC))


def _build_program(bias_val):
    import concourse.bass as bass
    import concourse.mybir as mybir
    import concourse.tile as tile

    _patch_drain()
    f32r = mybir.dt.float32r
    f32 = mybir.dt.float32

    nc = bass.Bass()
    xr = nc.declare_dram_parameter("xr", [XR_BANDS * 128, GROUPS], f32r, isOutput=False)
    wa = nc.declare_dram_parameter("wa", [128, NPASS * 128], f32r, isOutput=False)
    out = nc.declare_dram_parameter("out", [NBAND * 128, NG], f32, isOutput=True)

    with tile.TileContext(nc) as tc:
        with (
            tc.tile_pool(name="const", bufs=1) as constp,
            tc.tile_pool(name="psum", bufs=4, space="PSUM") as psp,
            tc.tile_pool(name="op", bufs=4) as outp,
        ):
            # Weights and output stores ride the Activation engine's HWDGE
            # ring; the SP ring is reserved for input tiles so the first x
            # band is the first transfer in its queue.
            wts = []
            for i in range(NPASS):
                wt = constp.tile([128, 128], f32r, tag=f"wa{i}")
                nc.scalar.dma_start(out=wt[:, :], in_=wa[:, 128 * i : 128 * (i + 1)])
                wts.append(wt)
            xts = []
            for t in range(XR_BANDS):
                xt = constp.tile([128, GROUPS], f32r, tag=f"x{t}")
                nc.sync.dma_start(out=xt[:, :], in_=xr[128 * t : 128 * (t + 1), :])
                xts.append(xt)

            for t in range(NBAND):
                ps = psp.tile([128, NG], f32, tag="ps")
                for idx in range(NPASS):
                    a, b = divmod(idx, 3)
                    nc.tensor.matmul(
                        ps[:, :],
                        wts[idx][:, :],
                        xts[t + a][:, b : b + NG],
                        start=(idx == 0),
                        stop=(idx == NPASS - 1),
                    )
                ot = outp.tile([128, NG], f32, tag="ot")
                nc.vector.tensor_scalar_add(ot[:, :], ps[:, :], bias_val)
                nc.scalar.dma_start(
                    out=out[128 * t : 128 * (t + 1), :], in_=ot[:, :]
                )

    _split_multi_waits(nc)
    return nc


def kernel(x, weight, bias):
    global LAST_RESULT
    from concourse.bass_utils import run_bass_kernel_spmd

    x = np.ascontiguousarray(np.asarray(x, dtype=np.float32))
    weight = np.asarray(weight, dtype=np.float32)
    bias = np.asarray(bias, dtype=np.float32)

    # Host-side zero padding: PAD on top/left, plus enough extra rows/cols
    # that every core's fixed-size slice stays in bounds.
    xpad = np.zeros((XPAD_R, XPAD_C), np.float32)
    xpad[PAD : PAD + H, PAD : PAD + W] = x
    A = _make_stationaries(weight)

    nc = _build_program(float(bias[0]))
    in_maps = []
    for c in range(NCORES):
        sl = xpad[ROWS_PC * c : ROWS_PC * c + XR_BANDS * BR]  # [528, 4128]
        xrc = (
            sl.reshape(XR_BANDS, BR, GROUPS, BC)
            .transpose(0, 1, 3, 2)
            .reshape(XR_BANDS * 128, GROUPS)
        )
        in_maps.append({"xr": np.ascontiguousarray(xrc), "wa": A})
    res = run_bass_kernel_spmd(
        nc,
        in_maps,
        list(range(NCORES)),
        trace=bool(os.environ.get("CONV_TRACE")),
    )
    LAST_RESULT = res

    full = np.empty((NCORES * ROWS_PC, NG * BC), np.float32)
    for c in range(NCORES):
        oc = res.results[c]["out"]  # [NBAND*128, NG]
        full[ROWS_PC * c : ROWS_PC * (c + 1)] = (
            oc.reshape(NBAND, BR, BC, NG)
            .transpose(0, 1, 3, 2)
            .reshape(ROWS_PC, NG * BC)
        )
    return np.ascontiguousarray(full[:OUT, :OUT]).astype(np.float32)
